# revision 11
# baseline (speedup 1.0000x reference)
"""Causal multi-head attention kernel for Trainium2, 8 NeuronCores.

Problem: x[4,2048,1024] fp32, Wq/Wk/Wv/Wo[1024,1024], bo[1024].
  y = softmax(causal(Q K^T)/sqrt(64)) V @ Wo + bo, H=16 heads of D=64.

Sharding (per hint): data-parallel over batch (4) x tensor-parallel over
heads (2 groups of 8). Core c handles batch c//2, heads (c%2)*8..+8:
Wq/Wk/Wv column-sharded, Wo row-sharded, pairwise ReduceScatter after
out_proj (each pair core returns half the query rows). One program runs
SPMD on all 8 cores; all per-core divergence comes from input data.

Host<->device I/O is the dominant per-dispatch cost in this environment
(every external tensor is re-staged per execution), so I/O is minimized:
  - all inputs/outputs staged as fp16 (PE runs fp16 at full rate with
    fp32 PSUM accumulation; ~1e-3 output rel err, well within budget);
  - x is de-duplicated across each TP pair: each core stages only half
    of x[b]^T and the pair AllGathers on device;
  - each TP half's weight set (Wq/Wk/Wv column half + Wo row half,
    2M fp16 elements flat) is staged in quarters and AllGathered across
    the 4 same-half cores ([[0,2,4,6],[1,3,5,7]]) so all cores address
    identical offsets;
  - output is pairwise ReduceScattered so each core returns only its
    1024 query rows as fp16.
Device-side collectives move HBM->HBM and are latency-cheap here.

Per-core compute layout (unchanged from the f32r version, fp16 operands):
  - x^T tiles feed every projection matmul directly (no on-device
    transposes); Q^T/K^T stored [128, S] per head-pair.
  - Scores computed transposed per (128-k-block x 512-q-tile); the two
    heads run as row-packed concurrent K=64 matmuls into adjacent PSUM
    banks; exp on ACT reads both banks [128,1024] in one instruction;
    causal masking multiplies the diagonal-band blocks by slices of one
    precomputed step-pattern ("mega-mask").
  - AV accumulated transposed with a ones-column per V block providing
    softmax denominators (normalization deferred to after AV).
  - Normalization via reciprocal + K=1 ones-matmul partition broadcast
    (kept in f32r: fp16 reciprocals of large denominators would hit
    subnormals); the normalized A^T is exactly the out-proj lhsT layout.
  - Out-proj accumulates 4 head-pair chunks + a K=1 bias matmul
    (bo/2 per TP rank), then DMA to DRAM, pairwise ReduceScatter.

Requires bacc.Bacc (not bass.Bass): its generate_event_semaphores pass
splits multi-waits (TRN2 allows one sync wait per instruction).
"""

import numpy as np

import concourse.bass as bass
from concourse import bacc
import concourse.mybir as mybir
import concourse.tile as tile
from concourse.bass_utils import run_bass_kernel_spmd

B, S, E, H, D = 4, 2048, 1024, 16, 64
ESH = 512           # per-core E shard (8 heads x 64)
SH = 1024           # per-core staged s-half of x
HP = 4              # head pairs per core
NJ, QTW = 4, 512    # q tiles
NKB, KBW = 16, 128  # k blocks
WQTR = E * ESH      # flat weight-quarter: 1/4 of one TP half's weight set

fp32 = mybir.dt.float32
f32r = mybir.dt.float32r
fp16 = mybir.dt.float16
DT = fp16           # matmul operand / staging dtype
AF = mybir.ActivationFunctionType

PAIRS = [[0, 1], [2, 3], [4, 5], [6, 7]]
HALVES = [[0, 2, 4, 6], [1, 3, 5, 7]]


def _body(tc, io):
    nc = tc.nc

    # ------- staging: gather x halves (pairs) + weight quarters (halves) -------
    dram_cm = tc.tile_pool(name="dram", bufs=1, space="DRAM")
    dram = dram_cm.__enter__()
    xtfull = dram.tile([2, E, SH], DT)
    wfull = dram.tile([4 * WQTR], DT)
    ypart = dram.tile([S, E], DT)
    yrs = dram.tile([S // 2, E], DT)
    # collectives may not read I/O tensors: bounce through internal DRAM.
    # All per-core inputs arrive as one flat blob [xth | wsh | bo] to avoid
    # per-tensor staging overhead.
    blob = io["blob"].ap()
    xth_i = dram.tile([E, SH], DT)
    wsh_i = dram.tile([WQTR], DT)
    nc.sync.dma_start(out=xth_i,
                      in_=blob[0:E * SH].rearrange("(e s) -> e s", s=SH))
    nc.sync.dma_start(out=wsh_i, in_=blob[E * SH:E * SH + WQTR])
    nc.gpsimd.collective_compute(
        "AllGather", mybir.AluOpType.bypass, replica_groups=PAIRS,
        ins=[xth_i.opt()], outs=[xtfull.opt()])
    nc.gpsimd.collective_compute(
        "AllGather", mybir.AluOpType.bypass, replica_groups=HALVES,
        ins=[wsh_i.opt()], outs=[wfull.opt()])

    const_cm = tc.tile_pool(name="const", bufs=1)
    const = const_cm.__enter__()
    ones_st = const.tile([33, 64], fp32, tag="ones_st")
    nc.vector.memset(ones_st, 1.0)
    ones = const.tile([33, 64], f32r, tag="ones")
    nc.vector.tensor_copy(ones, ones_st)
    oner_st = const.tile([1, 128], fp32, tag="oner_st")
    nc.vector.memset(oner_st, 1.0)
    onecol_st = const.tile([128, 8], fp32, tag="onecol_st")
    nc.vector.memset(onecol_st, 1.0)
    ones_row = const.tile([1, 128], DT, tag="ones_row")
    nc.vector.tensor_copy(ones_row, oner_st)
    # mega-mask M[kp,u] = (u >= kp+384); diag pattern r slice = [384-128r:+512]
    # generated on device: iota = u - kp - 384 >= 0 keeps 1.0, else 0.0
    masks = const.tile([128, 896], DT, tag="masks")
    nc.gpsimd.memset(masks, 1.0)
    nc.gpsimd.affine_select(
        out=masks, in_=masks, compare_op=mybir.AluOpType.is_ge, fill=0.0,
        base=-384, channel_multiplier=-1, pattern=[[1, 896]])
    bo_dt = const.tile([1, E], DT, tag="bo_dt")
    nc.sync.dma_start(out=bo_dt,
                      in_=io["blob"].ap()[E * SH + WQTR:E * SH + WQTR + E][None, :])
    # Wo stays resident through phase 3; Wq/Wk/Wv live in a phase-1 pool
    wo_sb = const.tile([128, 4, E], DT, tag="wo")
    nc.sync.dma_start(
        out=wo_sb,
        in_=wfull[3 * WQTR:4 * WQTR].rearrange("(c p e) -> p c e", p=128, e=E))

    kv_cm = tc.tile_pool(name="kv", bufs=1)
    kv = kv_cm.__enter__()
    qt_sb = [kv.tile([128, S], DT, tag=f"qt{hp}", name=f"qt{hp}") for hp in range(HP)]
    kt_sb = [kv.tile([128, S], DT, tag=f"kt{hp}", name=f"kt{hp}") for hp in range(HP)]
    v_sb = [kv.tile([128, 8, 65], DT, tag=f"v{kb}", name=f"v{kb}") for kb in range(NKB)]

    # ---------------- phase 1: projections ----------------
    wpool_cm = tc.tile_pool(name="wpool", bufs=1)
    wpool = wpool_cm.__enter__()
    wq_sb = wpool.tile([128, 8, ESH], DT, tag="wq")
    wk_sb = wpool.tile([128, 8, ESH], DT, tag="wk")
    wv_sb = wpool.tile([128, 8, ESH], DT, tag="wv")
    for t, mi in ((wq_sb, 0), (wk_sb, 1), (wv_sb, 2)):
        nc.sync.dma_start(
            out=t,
            in_=wfull[mi * WQTR:(mi + 1) * WQTR].rearrange(
                "(e p c) -> p e c", p=128, c=ESH))

    xpool_cm = tc.tile_pool(name="xpool", bufs=2)
    xpool = xpool_cm.__enter__()
    ps1_cm = tc.tile_pool(name="ps1", bufs=1, space="PSUM")
    ps1 = ps1_cm.__enter__()

    for st_i in range(4):
        xhalf, scol = st_i // 2, (st_i % 2) * 512
        ssl = slice(st_i * 512, (st_i + 1) * 512)
        # x^T page for this s-tile: [128, 8 E-chunks, 512]
        xt = xpool.tile([128, 8, 512], DT, tag="xt")
        nc.sync.dma_start(
            out=xt,
            in_=xtfull[xhalf].rearrange("(e p) s -> p e s", p=128)[:, :, scol:scol + 512])
        for hp in range(HP):
            psq = ps1.tile([128, 512], fp32, tag="pj", bufs=3)
            for e in range(8):
                nc.tensor.matmul(psq, wq_sb[:, e, hp * 128:(hp + 1) * 128],
                                 xt[:, e, :], start=(e == 0), stop=(e == 7))
            nc.vector.tensor_copy(qt_sb[hp][:, ssl], psq)
            psk = ps1.tile([128, 512], fp32, tag="pj", bufs=3)
            for e in range(8):
                nc.tensor.matmul(psk, wk_sb[:, e, hp * 128:(hp + 1) * 128],
                                 xt[:, e, :], start=(e == 0), stop=(e == 7))
            nc.vector.tensor_copy(kt_sb[hp][:, ssl], psk)
        for sb in range(4):
            psv = ps1.tile([128, 512], fp32, tag="pj", bufs=3)
            for e in range(8):
                nc.tensor.matmul(psv, xt[:, e, sb * 128:(sb + 1) * 128],
                                 wv_sb[:, e, :], start=(e == 0), stop=(e == 7))
            kb = st_i * 4 + sb
            nc.vector.tensor_copy(
                v_sb[kb][:, :, 0:64],
                psv.rearrange("p (h d) -> p h d", h=8))
            nc.vector.tensor_copy(
                v_sb[kb][:, :, 64:65],
                onecol_st.rearrange("p (h o) -> p h o", o=1))

    ps1_cm.__exit__(None, None, None)
    xpool_cm.__exit__(None, None, None)
    wpool_cm.__exit__(None, None, None)

    # -------- phase 2+3: attention, out-proj, reduce-scatter --------
    apool_cm = tc.tile_pool(name="apool", bufs=1)
    apool = apool_cm.__enter__()
    at_sb = [apool.tile([128, S], DT, tag=f"at{hp}", name=f"at{hp}") for hp in range(HP)]

    slabs_cm = tc.tile_pool(name="slabs", bufs=3)
    slabs = slabs_cm.__enter__()
    rpool_cm = tc.tile_pool(name="rpool", bufs=2)
    rpool = rpool_cm.__enter__()
    ypool_cm = tc.tile_pool(name="ypool", bufs=3)
    ypool = ypool_cm.__enter__()
    ps2_cm = tc.tile_pool(name="ps2", bufs=1, space="PSUM")
    ps2 = ps2_cm.__enter__()

    for j in range(NJ):
        jsl = slice(j * QTW, (j + 1) * QTW)
        for hp in range(HP):
            # the two heads accumulate in separate base-0 PSUM tiles; head
            # B's normalized rows reach at_sb partitions 64-127 via a
            # partition-shifting SBUF->SBUF DMA afterwards.
            avA = ps2.tile([65, 512], fp32, tag="avA", bufs=1)
            avB = ps2.tile([65, 512], fp32, tag="avB", bufs=1)
            kmax = 4 * j + 4
            for kb in range(kmax):
                ksl = slice(kb * KBW, (kb + 1) * KBW)
                sc = ps2.tile([128, 1024], fp32, tag="sc", bufs=2)
                nc.tensor.matmul(sc[:, 0:512], kt_sb[hp][0:64, ksl],
                                 qt_sb[hp][0:64, jsl], start=True, stop=True)
                nc.tensor.matmul(sc[:, 512:1024], kt_sb[hp][64:128, ksl],
                                 qt_sb[hp][64:128, jsl], start=True, stop=True)
                slab = slabs.tile([128, 1024], DT, tag="slab")
                nc.scalar.activation(slab, sc, AF.Exp, bias=0.0, scale=0.125)
                r = kb - 4 * j
                if r >= 0:
                    msl = slice(384 - 128 * r, 384 - 128 * r + 512)
                    nc.vector.tensor_mul(slab[:, 0:512], slab[:, 0:512],
                                         masks[:, msl])
                    nc.vector.tensor_mul(slab[:, 512:1024], slab[:, 512:1024],
                                         masks[:, msl])
                first, last = kb == 0, kb == kmax - 1
                nc.tensor.matmul(avA, v_sb[kb][:, 2 * hp, :],
                                 slab[:, 0:512], start=first, stop=last)
                nc.tensor.matmul(avB, v_sb[kb][:, 2 * hp + 1, :],
                                 slab[:, 512:1024], start=first, stop=last)
            recipA = rpool.tile([1, 512], f32r, tag="recipA")
            recipB = rpool.tile([1, 512], f32r, tag="recipB")
            with nc.allow_low_precision(reason="f32r recip, ~1e-6 rel err"):
                nc.vector.reciprocal(recipA, avA[64:65, :])
                nc.vector.reciprocal(recipB, avB[64:65, :])
            bcA = ps2.tile([64, 512], fp32, tag="bcA", bufs=1)
            bcB = ps2.tile([64, 512], fp32, tag="bcB", bufs=1)
            nc.tensor.matmul(bcA, ones[0:1, :], recipA, start=True, stop=True)
            nc.tensor.matmul(bcB, ones[0:1, :], recipB, start=True, stop=True)
            # DVE reads at most one PSUM operand: stage bc in SBUF
            bcA_sb = rpool.tile([64, 512], f32r, tag="bcA_sb")
            nc.vector.tensor_copy(bcA_sb, bcA)
            bcB_sb = rpool.tile([64, 512], f32r, tag="bcB_sb")
            nc.vector.tensor_copy(bcB_sb, bcB)
            nc.vector.tensor_mul(at_sb[hp][0:64, jsl], avA[0:64, :], bcA_sb)
            atB = rpool.tile([64, 512], DT, tag="atB")
            nc.vector.tensor_mul(atB, avB[0:64, :], bcB_sb)
            nc.sync.dma_start(out=at_sb[hp][64:128, jsl], in_=atB)
        # out-proj for q-tile j (+ bias via K=1 accumulate; bo pre-halved)
        for qs in range(4):
            q0 = j * QTW + qs * 128
            yp = ps2.tile([128, 1024], fp32, tag="sc", bufs=2)
            for half in range(2):
                hsl = slice(half * 512, (half + 1) * 512)
                for hp in range(HP):
                    nc.tensor.matmul(
                        yp[:, hsl], at_sb[hp][:, q0:q0 + 128],
                        wo_sb[:, hp, hsl], start=(hp == 0), stop=False)
                nc.tensor.matmul(yp[:, hsl], ones_row, bo_dt[0:1, hsl],
                                 start=False, stop=True)
            ysb = ypool.tile([128, E], DT, tag="ysb")
            nc.vector.tensor_copy(ysb, yp)
            nc.sync.dma_start(out=ypart[q0:q0 + 128, :], in_=ysb)
    nc.gpsimd.collective_compute(
        "ReduceScatter", mybir.AluOpType.add,
        replica_groups=PAIRS,
        ins=[ypart.opt()], outs=[yrs.opt()],
    )
    nc.sync.dma_start(out=io["y"].ap(), in_=yrs)

    for cm in (ps2_cm, ypool_cm, rpool_cm, slabs_cm, apool_cm,
               kv_cm, const_cm, dram_cm):
        cm.__exit__(None, None, None)


def build():
    nc = bacc.Bacc("TRN2", target_bir_lowering=False, debug=False,
                   num_devices=8)
    io = {
        "blob": nc.dram_tensor("blob", [E * SH + WQTR + E], fp16,
                               kind="ExternalInput"),
        "y": nc.dram_tensor("y", [S // 2, E], fp16, kind="ExternalOutput"),
    }
    with tile.TileContext(nc) as tc:
        _body(tc, io)
    nc.finalize()
    return nc


def make_in_maps(x, Wq, Wk, Wv, Wo, bo):
    """Shard full inputs into the 8 per-core input maps."""
    x = np.asarray(x, dtype=np.float32)
    Wq, Wk, Wv, Wo = (np.asarray(w, dtype=np.float32) for w in (Wq, Wk, Wv, Wo))
    bo = np.asarray(bo, dtype=np.float32)
    bo_h = (bo * 0.5).astype(np.float16)
    # flat per-TP-half weight set: [wq_half | wk_half | wv_half | wo_half]
    whalf = []
    for g in range(2):
        csl = slice(g * ESH, (g + 1) * ESH)
        whalf.append(np.concatenate(
            [Wq[:, csl].ravel(), Wk[:, csl].ravel(),
             Wv[:, csl].ravel(), Wo[csl, :].ravel()]).astype(np.float16))
    in_maps = []
    for c in range(8):
        b, g = c // 2, c % 2
        xth = np.ascontiguousarray(
            x[b, g * SH:(g + 1) * SH, :].T.astype(np.float16))
        in_maps.append({
            "blob": np.concatenate(
                [xth.ravel(), whalf[g][b * WQTR:(b + 1) * WQTR], bo_h]),
        })
    return in_maps


def kernel(x, Wq, Wk, Wv, Wo, bo):
    nc = build()
    in_maps = make_in_maps(x, Wq, Wk, Wv, Wo, bo)
    res = run_bass_kernel_spmd(nc, in_maps, core_ids=list(range(8)))
    y = np.empty((B, S, E), dtype=np.float32)
    for b in range(B):
        y[b, 0:SH] = res.results[2 * b]["y"].astype(np.float32)
        y[b, SH:S] = res.results[2 * b + 1]["y"].astype(np.float32)
    return y
